# revision 2
# baseline (speedup 1.0000x reference)
"""AttentionOverlapLoss Trainium2 kernel.

Reference (per image, attn_map (B,224,224) f32, bboxes (B,4) int):
    a    = (x - mn) / (mx - mn + eps)     # min-max normalize
    a    = a / (sum(a) + eps)             # sum-to-one
    loss = mean_b( sum(a * (1 - boxmask)) )

Algebra: with S = sum(x), Sbox = sum(x in box), N = H*W, Nbox = box area,
d = mx - mn + eps:
    loss_i = (sumA - sumAbox)/(sumA + eps),  sumA = (S - N*mn)/d,
    sumAbox = (Sbox - Nbox*mn)/d
d cancels between numerator and denominator up to the 1e-8 eps (~1e-12
relative), so mx is never needed and the host evaluates with d = 1.
The device computes, per image: mn, S, and per-column box-row sums
(the column box mask + final formula are applied on the host, which is
exact).

Device strategy (pure data parallel, batch sharded 8 ways; 256 images and
51.4 MB of attn_map per core — HBM-bandwidth bound):

  * Layout: flat local row r = g*1792 + 14*p + t  (g: 32 groups, p: 128
    partitions, t: 14 rows).  A partition line of one group is 14
    consecutive rows = 12544 B contiguous HBM; a group is 1.6 MB
    contiguous.  Each partition belongs to exactly one image per group
    (img = g*8 + p//16, within-image row = 14*(p%16) + t).
  * The group load is a single SWDGE (gpsimd) DMA that casts f32 -> bf16
    in flight — no engine time spent on the cast, and the SBUF write side
    halves.  Runs at ~334 GB/s of HBM read.
  * VectorE: min over each group via a 5-op bf16 halving tree
    (3136 -> 1568 -> 784 -> 392 -> 196 -> 1) -> mins[128, 32].
  * On-device indicator generation (no mask streaming): from a constant
    tval[p, t, j] = t if j == p//16 else 1e9 and per-(partition, group)
    bounds lo/hi (host folds the row base 14*(p%16) into y1/y2), three
    small DVE ops build the box half of the stationary:
        rl[:, :, 8:16] = (tval >= lo_g) * (tval <= hi_g)
    into two ping-pong tiles whose onehot half (cols 0..7) is written once.
  * TensorE: per t one matmul psum[16, 224] += rl[:, t, :].T @ xb[:, t, :]
    accumulated over the 14 t's: rows 0..7 = per-image total column sums,
    rows 8..15 = row-box-masked column sums.  ScalarE copies psum to SBUF.
  * Host: mn/S/Sbox reduction, column box mask, loss formula in f64.
"""

import contextlib

import numpy as np
import ml_dtypes

B, H, W = 2048, 224, 224
NCORES = 8
BL = B // NCORES            # 256 images per core
RPC = BL * H                # 57344 flat rows per core
P = 128
TPG = 14                    # rows per partition per group
NG = RPC // (P * TPG)       # 32 groups
IPG = BL // NG              # 8 images per group
NJ = 2 * IPG                # 16 stationary columns (totals | box)
NPIX = H * W
EPS = 1e-8

PROFILE = False
LAST_RESULT = None

_compiled = None


def build(loop_k=1, parts="all"):
    """loop_k > 1 wraps the 32-group body in a hardware For_i loop
    (benchmarking only); parts: ablation knob ("dma", "dma_trees",
    "dma_pe", "all")."""
    import concourse.bacc as bacc
    import concourse.bass as bass
    import concourse.mybir as mybir
    from concourse import tile

    f32 = mybir.dt.float32
    bf16 = mybir.dt.bfloat16

    nc = bacc.Bacc("TRN2", target_bir_lowering=False, debug=False,
                   num_devices=NCORES)
    x = nc.declare_dram_parameter("x", [BL, H, W], f32, isOutput=False)
    auxf = nc.declare_dram_parameter("auxf", [P, 2 * NG + TPG * IPG], f32,
                                     isOutput=False)
    # [onehot | zeros] per t: init image for the ping-pong rl tiles
    auxb = nc.declare_dram_parameter("auxb", [P, TPG * NJ], bf16,
                                     isOutput=False)
    mins_o = nc.declare_dram_parameter("mins", [P, NG], f32, isOutput=True)
    ps_o = nc.declare_dram_parameter("ps", [NJ, NG * W], f32, isOutput=True)

    do_pe = parts in ("all", "dma_pe")
    do_trees = parts in ("all", "dma_trees")

    with tile.TileContext(nc) as tc:
        with (
            tc.tile_pool(name="const", bufs=1) as constp,
            tc.tile_pool(name="xb", bufs=4) as xbp,
            tc.tile_pool(name="tree", bufs=2) as treep,
            tc.tile_pool(name="rl", bufs=2) as rlp,
            tc.tile_pool(name="psum", bufs=2,
                         space=bass.MemorySpace.PSUM) as psump,
        ):
            minstat = sbps = auxft = rlab = None
            if do_pe:
                sbps = constp.tile([NJ, NG * W], f32, tag="sbps")
                auxft = constp.tile([P, 2 * NG + TPG * IPG], f32, tag="auxft")
                nc.sync.dma_start(auxft[:], auxf[:])
                rl0 = constp.tile([P, TPG, NJ], bf16, tag="rl0")
                rl1 = constp.tile([P, TPG, NJ], bf16, tag="rl1")
                rlab = [rl0, rl1]
                for rt in rlab:
                    nc.sync.dma_start(
                        rt[:], auxb[:].rearrange("p (t j) -> p t j", j=NJ))
            if do_trees:
                minstat = constp.tile([P, NG], f32, tag="minstat")
            xview = (x[:].rearrange("b h w -> (b h) w")
                         .rearrange("(g p r) w -> p g (r w)", g=NG, p=P))

            def group_body(g):
                xb = xbp.tile([P, TPG, W], bf16, tag="xb")
                nc.gpsimd.dma_start(
                    xb[:],
                    xview[:, g:g + 1, :].rearrange("p g (r w) -> (p g) r w",
                                                   w=W))
                if parts == "dma":
                    return
                if do_pe:
                    # box half of the stationary: (tval >= lo)*(tval <= hi)
                    tval = (auxft[:, 2 * NG:2 * NG + TPG * IPG]
                            .rearrange("p (t j) -> p t j", j=IPG))
                    lo_g = auxft[:, g:g + 1]
                    hi_g = auxft[:, NG + g:NG + g + 1]
                    rl = rlab[g % 2]
                    ge = rlp.tile([P, TPG, IPG], bf16, tag="ge")
                    le = rlp.tile([P, TPG, IPG], bf16, tag="le")
                    nc.vector.tensor_scalar(ge[:], tval, lo_g, None,
                                            mybir.AluOpType.is_ge)
                    nc.vector.tensor_scalar(le[:], tval, hi_g, None,
                                            mybir.AluOpType.is_le)
                    nc.vector.tensor_tensor(rl[:, :, IPG:NJ], ge[:], le[:],
                                            mybir.AluOpType.mult)

                    pg = psump.tile([NJ, W], f32, tag="pg")
                    for t in range(TPG):
                        nc.tensor.matmul(pg[:], rl[:, t, :], xb[:, t, :],
                                         start=(t == 0), stop=(t == TPG - 1))
                    nc.scalar.copy(sbps[:, g * W:(g + 1) * W], pg[:])
                if do_trees:
                    op = mybir.AluOpType.min
                    xf = xb[:].rearrange("p a b -> p (a b)")
                    t1 = treep.tile([P, 1568], bf16, tag="t1min")
                    nc.vector.tensor_tensor(t1[:], xf[:, 0:1568],
                                            xf[:, 1568:3136], op)
                    t2 = treep.tile([P, 784], bf16, tag="t2min")
                    nc.vector.tensor_tensor(t2[:], t1[:, 0:784],
                                            t1[:, 784:1568], op)
                    t3 = treep.tile([P, 392], bf16, tag="t3min")
                    nc.vector.tensor_tensor(t3[:], t2[:, 0:392],
                                            t2[:, 392:784], op)
                    t4 = treep.tile([P, 196], bf16, tag="t4min")
                    nc.vector.tensor_tensor(t4[:], t3[:, 0:196],
                                            t3[:, 196:392], op)
                    nc.vector.tensor_reduce(
                        minstat[:, g:g + 1], t4[:],
                        axis=mybir.AxisListType.X, op=op)

            loop_cm = (tc.For_i(0, loop_k, 1) if loop_k > 1
                       else contextlib.nullcontext())
            with loop_cm:
                for g in range(NG):
                    group_body(g)

            if do_trees:
                nc.sync.dma_start(mins_o[:], minstat[:])
            if do_pe:
                nc.sync.dma_start(ps_o[:], sbps[:])

    nc.compile()
    return nc


def host_prep(x_np, bboxes):
    bb = np.asarray(bboxes).astype(np.int64)
    x1 = np.clip(bb[:, 0], 0, W - 1)
    y1 = np.clip(bb[:, 1], 0, H - 1)
    x2 = np.clip(bb[:, 2], 0, W - 1)
    y2 = np.clip(bb[:, 3], 0, H - 1)
    yy = np.arange(H)
    xx = np.arange(W)
    rbox = (yy[None, :] >= y1[:, None]) & (yy[None, :] <= y2[:, None])  # (B,H)
    cbox = (xx[None, :] >= x1[:, None]) & (xx[None, :] <= x2[:, None])  # (B,W)

    p_idx = np.arange(P)
    wbase = 14 * (p_idx % 16)                          # (P,)
    g_idx = np.arange(NG)
    onehot = (p_idx[:, None] // 16 ==
              np.arange(IPG)[None, :]).astype(np.float32)   # (P, IPG)
    # tval[p, t, j] = t if j == p//16 else 1e9
    tval = np.where(onehot[:, None, :] > 0,
                    np.arange(TPG, dtype=np.float32)[None, :, None],
                    np.float32(1e9)).reshape(P, TPG * IPG)
    # rl ping-pong init: [onehot | zeros] replicated per t
    auxb = np.zeros((P, TPG, NJ), np.float32)
    auxb[:, :, :IPG] = onehot[:, None, :]
    auxb = auxb.reshape(P, TPG * NJ)

    in_maps = []
    for c in range(NCORES):
        img = c * BL + g_idx[None, :] * IPG + p_idx[:, None] // 16  # (P, NG)
        lo = y1[img] - wbase[:, None]                  # (P, NG)
        hi = y2[img] - wbase[:, None]
        auxf = np.concatenate([lo, hi, tval], axis=1).astype(np.float32)
        in_maps.append({
            "x": np.ascontiguousarray(x_np[c * BL:(c + 1) * BL]),
            "auxf": np.ascontiguousarray(auxf),
            "auxb": np.ascontiguousarray(auxb).astype(ml_dtypes.bfloat16),
        })
    return in_maps, rbox, cbox


def host_combine(results, rbox, cbox):
    mn = np.empty(B)
    S = np.empty(B)
    Sbox = np.empty(B)
    for c in range(NCORES):
        r = results[c]
        # mins[p, g] -> image g*8 + p//16: group p as (j, q) = (p//16, p%16)
        mn[c * BL:(c + 1) * BL] = (r["mins"].reshape(IPG, 16, NG)
                                   .min(1).T.reshape(BL))
        ps = r["ps"].reshape(NJ, NG, W).astype(np.float64)
        S[c * BL:(c + 1) * BL] = ps[:IPG].sum(2).T.reshape(BL)
        cb = cbox[c * BL:(c + 1) * BL].reshape(NG, IPG, W)
        Sbox[c * BL:(c + 1) * BL] = (
            ps[IPG:].transpose(1, 0, 2) * cb).sum(2).reshape(BL)

    # d = mx-mn+eps cancels between sumA and sumAbox (to ~1e-12): use d = 1.
    nbox = rbox.sum(1).astype(np.float64) * cbox.sum(1).astype(np.float64)
    sumA = S - NPIX * mn
    sumAbox = Sbox - nbox * mn
    loss = (sumA - sumAbox) / (sumA + EPS)
    return np.float32(loss.mean())


def kernel(attn_map, bboxes):
    global _compiled, LAST_RESULT
    from concourse.bass_utils import run_bass_kernel_spmd

    if _compiled is None:
        _compiled = build()
    x_np = np.ascontiguousarray(np.asarray(attn_map, dtype=np.float32))
    in_maps, rbox, cbox = host_prep(x_np, bboxes)
    res = run_bass_kernel_spmd(_compiled, in_maps, list(range(NCORES)))
    LAST_RESULT = res
    return host_combine(res.results, rbox, cbox)
